# revision 10
# baseline (speedup 1.0000x reference)
"""MHA kernel for trn2: B=4, S=2048, D=1024, H=16 heads, dk=64.

Sharding: 8 cores = (batch b in 0..3) x (head-group g in 0..1, 8 heads each).
Per core: qh/kh/vh projections for its 512 output dims, flash-style attention
producing the full attn shard (8,2048,2048) plus partial out via Wo row-slice;
host sums the two head-group partials per batch.

NOTE: reference computes the query projection from k (quirk preserved): q unused.
Returns (out, attn) matching the reference tuple.
"""

import numpy as np

import bass_rust
import concourse.bass as bass
import concourse.tile as tile
from concourse import mybir
from concourse.bass import MemorySpace
from concourse.bass_utils import run_bass_kernel_spmd
from concourse.masks import make_identity
from concourse.tile import ScopedClock


def _patched_drain_and_barrier(self, tick_clock, wait_clock):
    # walrus rejects the stock drain when it carries >2 sem waits ("Too many
    # sync wait commands"); spread the waits across sync-engine nops instead.
    probe = self.nc.sync.nop(nofuse=True, hint="drain_wait0")
    wait_clock.add_sem_waits(probe.ins, ScopedClock({None: tick_clock.global_clock}))
    waits = list(probe.ins.sync_info.on_wait)
    probe.ins.sync_info = bass_rust.SyncInfo(on_wait=waits[:1], on_update=[])
    for i in range(1, len(waits)):
        n = self.nc.sync.nop(nofuse=True, hint=f"drain_wait{i}")
        n.ins.sync_info = bass_rust.SyncInfo(on_wait=[waits[i]], on_update=[])
    self.nc.sync.drain()
    self.nc.all_engine_barrier()
    assert self.sems is not None
    popped = self.nc._tile_sem_poison_stack.pop()
    assert popped is self._sem_poison
    self.nc.clear_and_free_semaphores(list(self.sems.allocated().values()))
    self.nc.all_engine_barrier()


tile.TileContext._drain_and_barrier = _patched_drain_and_barrier

P = 128
S = 2048
D = 1024
NH = 16
DK = 64
HG = 8          # heads per core
GD = 512        # projection dims per core
QT = S // P     # 16 query tiles
KB = S // 512   # 4 key blocks of 512
KC = S // P     # 16 key chunks of 128

F32 = mybir.dt.float32
F32R = mybir.dt.float32r

LAST_EXEC_NS = None


def _classify_mask(mask):
    """Per (qt, kb) block: 'V' all valid, 'I' all invalid, 'M' mixed.
    Returns (block_types [16][4], kb_end[16], mixed additive tiles, mixed index map)."""
    m = np.asarray(mask).astype(bool)
    types = [[None] * KB for _ in range(QT)]
    for qt in range(QT):
        for kb in range(KB):
            blk = m[qt * P:(qt + 1) * P, kb * 512:(kb + 1) * 512]
            types[qt][kb] = 'V' if blk.all() else ('I' if not blk.any() else 'M')
    kb_end = []
    for qt in range(QT):
        last = 0
        for kb in range(KB):
            if types[qt][kb] != 'I':
                last = kb + 1
        kb_end.append(last)
        # interior all-invalid blocks must be masked explicitly
        for kb in range(last):
            if types[qt][kb] == 'I':
                types[qt][kb] = 'M'
    tiles = []
    midx = {}
    for qt in range(QT):
        for kb in range(kb_end[qt]):
            if types[qt][kb] == 'M':
                blk = m[qt * P:(qt + 1) * P, kb * 512:(kb + 1) * 512]
                add = np.where(blk, np.float32(0.0), np.float32(-8e9))
                midx[(qt, kb)] = len(tiles)
                tiles.append(add)
    masks_np = (np.stack(tiles).astype(np.float32) if tiles
                else np.zeros((1, P, 512), np.float32))
    return types, kb_end, masks_np, midx


def _build(nc, types, kb_end, midx, n_mixed):
    kT = nc.declare_dram_parameter("kT", [D, S], F32R, isOutput=False)
    vT = nc.declare_dram_parameter("vT", [D, S], F32R, isOutput=False)
    wq = nc.declare_dram_parameter("wq", [D, GD], F32R, isOutput=False)
    wk = nc.declare_dram_parameter("wk", [D, GD], F32R, isOutput=False)
    wv = nc.declare_dram_parameter("wv", [D, GD], F32R, isOutput=False)
    wo = nc.declare_dram_parameter("wo", [GD, D], F32R, isOutput=False)
    bq = nc.declare_dram_parameter("bq", [GD], F32, isOutput=False)
    bk = nc.declare_dram_parameter("bk", [GD], F32, isOutput=False)
    bv = nc.declare_dram_parameter("bv", [GD], F32, isOutput=False)
    bo = nc.declare_dram_parameter("bo", [D], F32, isOutput=False)
    masks = nc.declare_dram_parameter("masks", [max(n_mixed, 1), P, 512], F32,
                                      isOutput=False)
    attn_out = nc.declare_dram_parameter("attn_out", [HG, S, S], F32R, isOutput=True)
    out_p = nc.declare_dram_parameter("out_p", [S, D], F32R, isOutput=True)

    with tile.TileContext(nc) as tc:
        with tc.tile_pool(name="persist", bufs=1) as persist, \
             tc.tile_pool(name="consts", bufs=1) as consts:
            qhT = persist.tile([P, 4, S], F32R)     # [d%128, d//128, s]
            khT = persist.tile([P, 4, S], F32R)
            vh = persist.tile([P, KC, GD], F32R)    # [s%128, s//128, dv]
            houtT = persist.tile([P, 4, S], F32R)   # [dh%128, dh//128, q]
            ident = consts.tile([P, P], F32R)
            # walrus rejects Memset on float32r; zero through an f32 view
            nc.gpsimd.memset(ident[:].bitcast(F32), 0.0)
            make_identity(nc, ident, nomemset=True)

            # ---- Phase 1a: qh/kh projections ----
            kT_r = kT[:].rearrange("(c p) s -> p c s", p=P)
            vT_r = vT[:].rearrange("(c p) s -> p c s", p=P)
            with tc.tile_pool(name="wpool", bufs=1) as wpool, \
                 tc.tile_pool(name="stream", bufs=2) as stream, \
                 tc.tile_pool(name="ppsum", bufs=4, space=MemorySpace.PSUM) as ppsum:
                wq_sb = wpool.tile([P, 8, GD], F32R)
                wk_sb = wpool.tile([P, 8, GD], F32R)
                nc.sync.dma_start(out=wq_sb, in_=wq[:].rearrange("(c p) n -> p c n", p=P))
                nc.sync.dma_start(out=wk_sb, in_=wk[:].rearrange("(c p) n -> p c n", p=P))
                bq_sb = wpool.tile([P, 4], F32)
                bk_sb = wpool.tile([P, 4], F32)
                nc.sync.dma_start(out=bq_sb, in_=bq[:].rearrange("(c p) -> p c", p=P))
                nc.sync.dma_start(out=bk_sb, in_=bk[:].rearrange("(c p) -> p c", p=P))

                for hb in range(8):  # s half-blocks of 256
                    s0 = hb * 256
                    kt = stream.tile([P, 8, 256], F32R)
                    nc.sync.dma_start(out=kt, in_=kT_r[:, :, s0:s0 + 256])
                    for w_sb, b_sb, outT in ((wq_sb, bq_sb, qhT), (wk_sb, bk_sb, khT)):
                        for dc in range(4):
                            ps = ppsum.tile([P, 256], F32)
                            for ec in range(8):
                                nc.tensor.matmul(
                                    ps, w_sb[:, ec, dc * P:(dc + 1) * P],
                                    kt[:, ec, :], start=(ec == 0), stop=(ec == 7))
                            nc.vector.tensor_scalar_add(
                                out=outT[:, dc, s0:s0 + 256], in0=ps,
                                scalar1=b_sb[:, dc:dc + 1])

            # ---- Phase 1b: vh projection ----
            with tc.tile_pool(name="wpoolv", bufs=1) as wpoolv, \
                 tc.tile_pool(name="streamv", bufs=2) as streamv, \
                 tc.tile_pool(name="vpsum", bufs=4, space=MemorySpace.PSUM) as vpsum:
                wv_sb = wpoolv.tile([P, 8, GD], F32R)
                nc.sync.dma_start(out=wv_sb, in_=wv[:].rearrange("(c p) n -> p c n", p=P))
                bv_sb = wpoolv.tile([P, GD], F32)
                nc.gpsimd.dma_start(
                    out=bv_sb,
                    in_=bass.AP(tensor=bv[:].tensor, offset=bv[:].offset,
                                ap=[[0, P]] + list(bv[:].ap)))
                for hb in range(8):
                    s0 = hb * 256
                    vt = streamv.tile([P, 8, 256], F32R)
                    nc.sync.dma_start(out=vt, in_=vT_r[:, :, s0:s0 + 256])
                    for ss in range(2):
                        ps = vpsum.tile([P, GD], F32)
                        for ec in range(8):
                            nc.tensor.matmul(
                                ps, vt[:, ec, ss * P:(ss + 1) * P],
                                wv_sb[:, ec, :], start=(ec == 0), stop=(ec == 7))
                        nc.vector.tensor_tensor(
                            vh[:, hb * 2 + ss, :], ps, bv_sb,
                            mybir.AluOpType.add)

            # ---- Phase 2: attention ----
            with tc.tile_pool(name="apool", bufs=3) as apool, \
                 tc.tile_pool(name="atp", bufs=1) as atp, \
                 tc.tile_pool(name="mpool", bufs=2) as mpool, \
                 tc.tile_pool(name="small", bufs=4) as small, \
                 tc.tile_pool(name="spsum", bufs=1, space=MemorySpace.PSUM) as spsum, \
                 tc.tile_pool(name="tpsum", bufs=2, space=MemorySpace.PSUM) as tpsum, \
                 tc.tile_pool(name="opsum", bufs=2, space=MemorySpace.PSUM) as opsum:
                attnT = atp.tile([P, KC, 256], F32R)
                for h in range(HG):
                    po = (h % 2) * DK
                    hc = h // 2
                    qh_h = qhT[po:po + DK, hc, :]
                    kh_h = khT[po:po + DK, hc, :]
                    for qt in range(QT):
                        vend = kb_end[qt] * 512
                        q0 = qt * P
                        at = apool.tile([P, S], F32R)
                        ps_s = spsum.tile([P, S], F32)
                        for kb in range(kb_end[qt]):
                            nc.tensor.matmul(
                                ps_s[:, kb * 512:(kb + 1) * 512],
                                qh_h[:, q0:q0 + P],
                                kh_h[:, kb * 512:(kb + 1) * 512],
                                start=True, stop=True)
                        for kb in range(kb_end[qt]):
                            if types[qt][kb] == 'M':
                                mt = mpool.tile([P, 512], F32)
                                nc.sync.dma_start(out=mt, in_=masks[midx[(qt, kb)]])
                                nc.vector.tensor_tensor(
                                    ps_s[:, kb * 512:(kb + 1) * 512],
                                    ps_s[:, kb * 512:(kb + 1) * 512],
                                    mt, mybir.AluOpType.add)
                        dn = small.tile([P, 1], F32)
                        r = small.tile([P, 1], F32)
                        nc.scalar.activation(
                            out=at[:, 0:vend], in_=ps_s[:, 0:vend],
                            func=mybir.ActivationFunctionType.Exp,
                            scale=0.125, accum_out=dn)
                        nc.vector.reciprocal(r, dn)
                        nc.vector.tensor_scalar_mul(at[:, 0:vend], at[:, 0:vend], r)
                        if vend < S:
                            nc.gpsimd.memset(at[:, vend:S].bitcast(F32), 0.0)
                        nc.sync.dma_start(out=attn_out[h, q0:q0 + P, :], in_=at)
                        # transposes into attnT for this half-qblock
                        hb2 = qt % 2
                        vend_hb = max(vend, kb_end[qt - 1] * 512) if hb2 else vend
                        # must cover union of both qts in the half-block
                        if hb2 == 0 and qt + 1 < QT:
                            vend_hb = max(vend, kb_end[qt + 1] * 512)
                        nkc4 = (vend_hb + 511) // 512
                        for kc4 in range(nkc4):
                            ps_t = tpsum.tile([P, 4, P], F32R)
                            for j in range(4):
                                kc = kc4 * 4 + j
                                nc.tensor.transpose(
                                    ps_t[:, j, :], at[:, kc * P:(kc + 1) * P], ident)
                            nc.vector.tensor_copy(
                                out=attnT[:, kc4 * 4:(kc4 + 1) * 4,
                                          hb2 * P:(hb2 + 1) * P],
                                in_=ps_t)
                        if hb2 == 1:
                            nkc = vend_hb // P
                            ps_o = opsum.tile([DK, 256], F32)
                            for i, kc in enumerate(range(nkc)):
                                nc.tensor.matmul(
                                    ps_o, vh[:, kc, h * DK:(h + 1) * DK],
                                    attnT[:, kc, :],
                                    start=(i == 0), stop=(i == nkc - 1))
                            hb = qt // 2
                            nc.vector.tensor_copy(
                                out=houtT[po:po + DK, hc, hb * 256:(hb + 1) * 256],
                                in_=ps_o)

            # ---- Phase 3: Wo ----
            with tc.tile_pool(name="wpool2", bufs=1) as wpool2, \
                 tc.tile_pool(name="otile", bufs=3) as otile, \
                 tc.tile_pool(name="wpsum", bufs=4, space=MemorySpace.PSUM) as wpsum:
                wo_sb = wpool2.tile([P, 4, D], F32R)
                nc.sync.dma_start(out=wo_sb, in_=wo[:].rearrange("(c p) n -> p c n", p=P))
                bo_sb = wpool2.tile([P, D], F32)
                nc.gpsimd.dma_start(
                    out=bo_sb,
                    in_=bass.AP(tensor=bo[:].tensor, offset=bo[:].offset,
                                ap=[[0, P]] + list(bo[:].ap)))
                for qt in range(QT):
                    q0 = qt * P
                    for nb in range(2):
                        ps = wpsum.tile([P, 512], F32)
                        for c in range(4):
                            nc.tensor.matmul(
                                ps, houtT[:, c, q0:q0 + P],
                                wo_sb[:, c, nb * 512:(nb + 1) * 512],
                                start=(c == 0), stop=(c == 3))
                        ot = otile.tile([P, 512], F32R)
                        nc.vector.tensor_tensor(
                            ot, ps, bo_sb[:, nb * 512:(nb + 1) * 512],
                            mybir.AluOpType.add)
                        nc.sync.dma_start(
                            out=out_p[q0:q0 + P, nb * 512:(nb + 1) * 512], in_=ot)

    # walrus enforces <=1 sem wait per instruction (2 for event sems); run the
    # Bacc legalization passes the plain-Bass BIR path skips.
    bass_rust.move_matmul_waits_to_ldweights(nc.m)
    bass_rust.generate_event_semaphores(nc)
    return nc


def kernel(q, k, v, mask, Wq, bq, Wk, bk, Wv, bv, Wo, bo, _trace=False):
    global LAST_EXEC_NS
    B = k.shape[0]
    types, kb_end, masks_np, midx = _classify_mask(mask)
    n_mixed = len(midx)

    nc = bass.Bass()
    _build(nc, types, kb_end, midx, n_mixed)

    zeros_bo = np.zeros_like(np.asarray(bo, dtype=np.float32))
    in_maps = []
    for core in range(8):
        b, g = core // 2, core % 2
        gsl = slice(g * GD, (g + 1) * GD)
        in_maps.append({
            "kT": np.ascontiguousarray(np.asarray(k[b], np.float32).T),
            "vT": np.ascontiguousarray(np.asarray(v[b], np.float32).T),
            "wq": np.ascontiguousarray(np.asarray(Wq, np.float32)[:, gsl]),
            "wk": np.ascontiguousarray(np.asarray(Wk, np.float32)[:, gsl]),
            "wv": np.ascontiguousarray(np.asarray(Wv, np.float32)[:, gsl]),
            "wo": np.ascontiguousarray(np.asarray(Wo, np.float32)[gsl, :]),
            "bq": np.ascontiguousarray(np.asarray(bq, np.float32)[gsl]),
            "bk": np.ascontiguousarray(np.asarray(bk, np.float32)[gsl]),
            "bv": np.ascontiguousarray(np.asarray(bv, np.float32)[gsl]),
            "bo": (np.asarray(bo, np.float32) if g == 0 else zeros_bo),
            "masks": masks_np,
        })

    if _trace:
        try:
            import profhook

            profhook.install()
        except ImportError:
            pass
    res = run_bass_kernel_spmd(nc, in_maps, list(range(8)), trace=_trace)
    LAST_EXEC_NS = res.exec_time_ns

    attn = np.empty((B, NH, S, S), np.float32)
    out = np.empty((B, S, D), np.float32)
    for core in range(8):
        b, g = core // 2, core % 2
        attn[b, g * HG:(g + 1) * HG] = res.results[core]["attn_out"]
    for b in range(B):
        out[b] = res.results[2 * b]["out_p"] + res.results[2 * b + 1]["out_p"]
    return out, attn
